# revision 1
# baseline (speedup 1.0000x reference)
"""BiMamba block Trainium2 kernel (v2).

Sharding: pure data-parallel over (direction, batch) = 2*4 = 8 units, one per
NeuronCore. Each core runs an identical Bass program computing a full Mamba
forward pass for one (batch, direction) sequence.

v2 scheduling改 vs baseline:
  - z-half of in_proj + its silu moved out of the serial front into the P5
    scan region (front = xi-half in_proj + x_proj + first dt only).
  - Activation-table thrash fixed: ACT work batched by function (Copy/Silu in
    P1, Exp-group then Ln-group in P4, Exp-only in P5 except one silu per blk).
  - dt computed once full-L per blk (softplus via Exp then Ln(1+e)), kept
    resident; dA = Exp(A_n*dt) on ACT per (blk, n, half).
  - conv as 4x tensor_scalar products + 2x shifted tensor_tensor adds
    (instead of 1x scalar_tensor_tensor chain).
  - u/hc multiplies split DVE (n < ND) / GPSIMD (n >= ND).
  - out_proj for each time-half runs overlapped inside the next scan half.

Layout: channels on partitions (16 blocks of 128), time on the free axis.
All 16-bit tensors are fp16. Scan decay dA is fp32 in the scan's internal
state (hardware); dA tiles themselves are fp32 to preserve baseline accuracy.
"""

from contextlib import ExitStack

import numpy as np

D_MODEL, D_STATE, D_CONV = 1024, 16, 4
D_INNER = 2048
DT_RANK = 64
B_SZ, SEQ = 4, 2048
NB = D_INNER // 128  # 16 channel blocks
HL = SEQ // 2        # 1024, time half
ND = 9               # u/hc with n < ND on DVE; rest GPSIMD

_CACHE = {}


def _pack_consts(conv_w, conv_b, dtb, Dp, A):
    # [128, 16*23] f32; per blk: A(16)|cw(4)|cb|dtb|Dp, rows = channel%128
    out = np.zeros((128, 16 * 23), np.float32)
    for blk in range(16):
        sl = slice(blk * 128, (blk + 1) * 128)
        out[:, blk * 23:blk * 23 + 16] = A[sl]
        out[:, blk * 23 + 16:blk * 23 + 20] = conv_w[sl]
        out[:, blk * 23 + 20] = conv_b[sl]
        out[:, blk * 23 + 21] = dtb[sl]
        out[:, blk * 23 + 22] = Dp[sl]
    return out


def _pad_xwT(xw):
    # xw: [96, 2048] -> transpose and pad to [2048, 112] with C at cols 96:112
    out = np.zeros((2048, 112), np.float16)
    xwT = xw.T.astype(np.float16)
    out[:, 0:80] = xwT[:, 0:80]
    out[:, 96:112] = xwT[:, 80:96]
    return out


def build_program():
    import concourse.bass as bass
    import concourse.bacc as bacc
    import concourse.tile as tile
    from concourse import mybir
    from concourse.masks import make_identity

    f16 = mybir.dt.float16
    f32 = mybir.dt.float32
    AF = mybir.ActivationFunctionType
    OP = mybir.AluOpType

    nc = bacc.Bacc()

    xT = nc.declare_dram_parameter("xT", [D_MODEL, SEQ], f16, isOutput=False)
    in_wT = nc.declare_dram_parameter("in_wT", [D_MODEL, 2 * D_INNER], f16, isOutput=False)
    XPW = 112  # x_proj out: dt_raw 0:64, B 64:80, pad, C 96:112
    xwT = nc.declare_dram_parameter("xwT", [D_INNER, XPW], f16, isOutput=False)
    dtwT = nc.declare_dram_parameter("dtwT", [DT_RANK, D_INNER], f16, isOutput=False)
    owT = nc.declare_dram_parameter("owT", [D_INNER, D_MODEL], f16, isOutput=False)
    CPW = 23
    consts_d = nc.declare_dram_parameter("consts_packed", [128, NB * CPW], f32, isOutput=False)
    y_out = nc.declare_dram_parameter("y", [D_MODEL, SEQ], f32, isOutput=True)

    # DRAM staging; stage_d packs xc | sz | dt per row so P5 can fetch all
    # per-block inputs in a single DMA (SP queue serializes DMAs; fewer and
    # wait-free ones matter).
    stage_d = nc.dram_tensor("stage_d", [D_INNER, 3 * SEQ], f16)
    yf_d = nc.dram_tensor("yf_d", [D_INNER, SEQ], f16)
    B_d = nc.dram_tensor("B_d", [D_STATE, SEQ], f16)
    C_d = nc.dram_tensor("C_d", [D_STATE, SEQ], f16)

    with tile.TileContext(nc) as tc, ExitStack() as ctx:
        consts = ctx.enter_context(tc.tile_pool(name="consts", bufs=1))

        I128 = consts.tile([128, 128], f16, tag="I128")
        make_identity(nc, I128)
        call = consts.tile([128, NB * CPW], f32, tag="call")
        nc.sync.dma_start(out=call, in_=consts_d[:, :])

        def A_col(blk, n):
            return call[:, blk * CPW + n:blk * CPW + n + 1]

        def cw_col(blk, k):
            return call[:, blk * CPW + 16 + k:blk * CPW + 16 + k + 1]

        def cb_col(blk):
            return call[:, blk * CPW + 20:blk * CPW + 21]

        def dtb_col(blk):
            return call[:, blk * CPW + 21:blk * CPW + 22]

        def Dp_col(blk):
            return call[:, blk * CPW + 22:blk * CPW + 23]

        dtwT_sb = consts.tile([DT_RANK, D_INNER], f16, tag="dtwT")
        nc.scalar.dma_start(out=dtwT_sb, in_=dtwT[:, :])
        dtraw = consts.tile([DT_RANK, SEQ], f16, tag="dtraw")
        carry_all = consts.tile([128, NB * D_STATE], f16, tag="carry_all")

        # xT stays resident through half 0 of P5 (z-half in_proj reads it);
        # allocated first so later pools release in LIFO order before it.
        xtpool = tc.alloc_tile_pool(name="xtpool", bufs=1)
        xT_sb = []
        for k in range(8):
            t = xtpool.tile([128, SEQ], f16, tag=f"xT{k}", name=f"xT{k}")
            nc.scalar.dma_start(out=t, in_=xT[k * 128:(k + 1) * 128, :])
            xT_sb.append(t)

        xwpool = tc.alloc_tile_pool(name="xwpool", bufs=1)  # released after P3
        xwT_sb = []
        for k in range(NB):
            t = xwpool.tile([128, XPW], f16, tag=f"xwT{k}", name=f"xwT{k}")
            nc.scalar.dma_start(out=t, in_=xwT[k * 128:(k + 1) * 128, :])
            xwT_sb.append(t)

        # ---- P1: xi-half in_proj + conv + silu ----
        xcpool = tc.alloc_tile_pool(name="xcpool", bufs=1)  # released after P3
        xc_sb = [None] * NB
        inwpool = tc.alloc_tile_pool(name="inwpool", bufs=1)  # released after P1
        inw_sb = []
        for k in range(8):
            t = inwpool.tile([128, D_INNER], f16, tag=f"inw{k}")
            nc.scalar.dma_start(out=t, in_=in_wT[k * 128:(k + 1) * 128, 0:D_INNER])
            inw_sb.append(t)

        with tc.tile_pool(name="p1w", bufs=2) as p1w, \
             tc.tile_pool(name="p1ps", bufs=2, space="PSUM") as p1ps:
            pending = []
            for blk in range(NB):
                for fn in pending:
                    fn()
                pending = []
                psum = p1ps.tile([128, SEQ], f32, tag="xz")
                for nt in range(4):
                    cs = slice(nt * 512, (nt + 1) * 512)
                    for k in range(8):
                        nc.tensor.matmul(
                            psum[:, cs],
                            lhsT=inw_sb[k][:, blk * 128:(blk + 1) * 128],
                            rhs=xT_sb[k][:, cs],
                            start=(k == 0), stop=(k == 7),
                        )
                xi_t = p1w.tile([128, SEQ], f16, tag="xi")
                nc.scalar.activation(out=xi_t, in_=psum, func=AF.Copy)
                # conv: acc = xi*cw3 + cb; acc[:,d:] += (xi*cw_k)[:, :L-d]
                acc = p1w.tile([128, SEQ], f16, tag="acc")
                nc.vector.tensor_scalar(
                    out=acc, in0=xi_t, scalar1=cw_col(blk, 3),
                    scalar2=cb_col(blk), op0=OP.mult, op1=OP.add,
                )
                for k in range(3):
                    d = 3 - k
                    tmp = p1w.tile([128, SEQ], f16, tag="cv")
                    nc.vector.tensor_scalar(
                        out=tmp, in0=xi_t, scalar1=cw_col(blk, k),
                        scalar2=None, op0=OP.mult,
                    )
                    nc.vector.tensor_tensor(
                        out=acc[:, d:], in0=tmp[:, :SEQ - d], in1=acc[:, d:],
                        op=OP.add,
                    )
                xc_t = xcpool.tile([128, SEQ], f16, tag=f"xc{blk}", name=f"xc{blk}")
                xc_sb[blk] = xc_t
                nc.scalar.activation(out=xc_t, in_=acc, func=AF.Silu)
                pending.append(lambda blk=blk, xc_t=xc_t: nc.sync.dma_start(
                    out=stage_d[blk * 128:(blk + 1) * 128, 0:SEQ], in_=xc_t))
            for fn in pending:
                fn()
        inwpool.release()

        # ---- P3: x_proj ----
        with tc.tile_pool(name="p3", bufs=1) as p3pool, \
             tc.tile_pool(name="pp3", bufs=1, space="PSUM") as pp3:
            psum_proj = pp3.tile([XPW, SEQ], f32, tag="proj")
            for nt in range(4):
                cs = slice(nt * 512, (nt + 1) * 512)
                for k in range(NB):
                    nc.tensor.matmul(
                        psum_proj[:, cs], lhsT=xwT_sb[k], rhs=xc_sb[k][:, cs],
                        start=(k == 0), stop=(k == NB - 1),
                    )
            nc.scalar.activation(out=dtraw, in_=psum_proj[0:DT_RANK, :], func=AF.Copy)
            B_sb = p3pool.tile([D_STATE, SEQ], f16, tag="Bs")
            C_sb = p3pool.tile([D_STATE, SEQ], f16, tag="Cs")
            nc.scalar.activation(out=B_sb, in_=psum_proj[64:80, :], func=AF.Copy)
            nc.scalar.activation(out=C_sb, in_=psum_proj[96:112, :], func=AF.Copy)
            nc.sync.dma_start(out=B_d[:, :], in_=B_sb)
            nc.sync.dma_start(out=C_d[:, :], in_=C_sb)
        xcpool.release()
        xwpool.release()

        # ---- P4: dt = softplus(dtw @ dtraw + dtb), full L, spilled to DRAM ----
        # Batched by activation function to avoid table churn: per group of 8
        # blocks, all Exp first (e = exp(v + dtb)), then all Ln (dt = ln(1+e)).
        with tc.tile_pool(name="p4e", bufs=4) as p4e, \
             tc.tile_pool(name="p4dt", bufs=2) as p4dt, \
             tc.tile_pool(name="pp4", bufs=2, space="PSUM") as pp4:
            for g in range(4):
                e_t = [None] * 4
                for j in range(4):
                    blk = g * 4 + j
                    psum_dt = pp4.tile([128, SEQ], f32, tag="pdt", name=f"pdt{blk}")
                    for nt in range(4):
                        cs = slice(nt * 512, (nt + 1) * 512)
                        nc.tensor.matmul(
                            psum_dt[:, cs],
                            lhsT=dtwT_sb[:, blk * 128:(blk + 1) * 128],
                            rhs=dtraw[:, cs], start=True, stop=True,
                        )
                    e_t[j] = p4e.tile([128, SEQ], f16, tag="e", name=f"e{blk}")
                    nc.scalar.activation(
                        out=e_t[j], in_=psum_dt, func=AF.Exp,
                        bias=dtb_col(blk), scale=1.0,
                    )
                dtp = []
                for j in range(4):
                    blk = g * 4 + j
                    for fn in dtp:
                        fn()
                    dtp = []
                    dt_full = p4dt.tile([128, SEQ], f16, tag="dt", name=f"dt{blk}")
                    nc.scalar.activation(
                        out=dt_full, in_=e_t[j], func=AF.Ln, bias=1.0, scale=1.0)
                    dtp.append(lambda blk=blk, dt_full=dt_full: nc.sync.dma_start(
                        out=stage_d[blk * 128:(blk + 1) * 128, 2 * SEQ:3 * SEQ],
                        in_=dt_full))
                for fn in dtp:
                    fn()

        # ---- P5: scan core, two time halves; z-half in_proj + silu runs
        # inside half 0; out_proj for half h runs inside half 1 / after. ----
        owT_sb = []

        def emit_p6(gt, p6y, ppo):
            # out_proj for global 512-col tile gt; single merged yf load
            gs = slice(gt * 512, (gt + 1) * 512)
            yf_all = p6y.tile([128, NB * 512], f16, tag="yfall", name=f"yfall{gt}")
            ysrc = bass.AP(tensor=yf_d, offset=gt * 512,
                           ap=[[SEQ, 128], [128 * SEQ, NB], [1, 512]])
            nc.sync.dma_start(out=yf_all, in_=ysrc)
            pend = []
            for m in range(8):
                psum_o = ppo.tile([128, 512], f32, tag="po", name=f"po{m}_{gt}")
                for k in range(NB):
                    nc.tensor.matmul(
                        psum_o, lhsT=owT_sb[k][:, m * 128:(m + 1) * 128],
                        rhs=yf_all[:, k * 512:(k + 1) * 512],
                        start=(k == 0), stop=(k == NB - 1),
                    )
                yo = p6y.tile([128, 512], f32, tag="yo", name=f"yo{m}_{gt}")
                nc.scalar.activation(out=yo, in_=psum_o, func=AF.Copy)
                for fn in pend:
                    fn()
                pend = [lambda m=m, yo=yo: nc.sync.dma_start(
                    out=y_out[m * 128:(m + 1) * 128, gs], in_=yo)]
            for fn in pend:
                fn()

        def emit_half(half, bc_pool, p5s, p5w, p5dA, ppy, p5u, p5hc, p5h,
                      zwpool=None, szpool=None, ppz=None, p6y=None, ppo=None):
            hs = slice(half * HL, (half + 1) * HL)
            B_bc = bc_pool.tile([128, D_STATE * HL], f16, tag="B_bc")
            C_bc = bc_pool.tile([128, D_STATE * HL], f16, tag="C_bc")
            for n0, n1 in ((ND, D_STATE), (0, ND)):
                for tens, bc in ((B_d, B_bc), (C_d, C_bc)):
                    srcap = bass.AP(tensor=tens, offset=n0 * SEQ + half * HL,
                                    ap=[[0, 128], [SEQ, n1 - n0], [1, HL]])
                    nc.sync.dma_start(out=bc[:, n0 * HL:n1 * HL], in_=srcap)
            pending = []
            zw4 = None
            POOL_NS = list(range(ND, D_STATE))
            N_ORDER = POOL_NS + list(range(ND))

            def load_cxd(blk):
                if half == 0:
                    cxd = p5s.tile([128, 2 * HL], f16, tag="cxd", bufs=3)
                    csrc = bass.AP(tensor=stage_d, offset=blk * 128 * 3 * SEQ + half * HL,
                                   ap=[[3 * SEQ, 128], [2 * SEQ, 2], [1, HL]])
                    nc.sync.dma_start(out=cxd, in_=csrc)
                    return cxd[:, 0:HL], cxd[:, HL:2 * HL]
                cxd = p5s.tile([128, 3 * HL], f16, tag="cxd3", bufs=2)
                csrc = bass.AP(tensor=stage_d, offset=blk * 128 * 3 * SEQ + half * HL,
                               ap=[[3 * SEQ, 128], [SEQ, 3], [1, HL]])
                nc.sync.dma_start(out=cxd, in_=csrc)
                return cxd, None

            def calc_dtxc(xc_t, dt_h):
                dtxc = p5w.tile([128, HL], f16, tag="dtxc", bufs=4 if half == 0 else 2)
                nc.vector.tensor_tensor(out=dtxc, in0=dt_h, in1=xc_t, op=OP.mult)
                return dtxc

            # prologue: prefetch PD blocks of inputs + dtxc. Cross-engine
            # deps are per-engine monotonic counters, and Pool runs a full
            # block ahead of DVE, so dtxc must be computed 2 blocks early.
            PD = 2 if half == 0 else 1
            fifo = []

            def prefetch(b):
                if half == 0:
                    xc_p, dt_p = load_cxd(b)
                    sz_p = None
                else:
                    cxd_p, _ = load_cxd(b)
                    xc_p, sz_p, dt_p = (cxd_p[:, 0:HL], cxd_p[:, HL:2 * HL],
                                        cxd_p[:, 2 * HL:3 * HL])
                fifo.append((xc_p, sz_p, dt_p, calc_dtxc(xc_p, dt_p)))

            sz_fifo = []
            new_pending = []

            def emit_z(zb):
                # z-half in_proj for block zb + silu -> sz (full L); emitted
                # two blocks early so PE's in-order queue never delays the
                # nsum matmuls that gate hc buffer recycling
                zrs = slice(zb * 128, (zb + 1) * 128)
                zw4 = zwpool.tile([128, 8 * 128], f16, tag="zw4")
                zsrc = bass.AP(
                    tensor=in_wT, offset=D_INNER + zb * 128,
                    ap=[[2 * D_INNER, 128], [128 * 2 * D_INNER, 8], [1, 128]])
                nc.sync.dma_start(out=zw4, in_=zsrc)
                sz_full = szpool.tile([128, SEQ], f16, tag="szf", bufs=2)
                for seg in range(2):
                    psum_z = ppz.tile([128, HL], f32, tag="pz")
                    for s5 in range(2):
                        zcs = slice(s5 * 512, (s5 + 1) * 512)
                        xcs = slice(seg * HL + s5 * 512,
                                    seg * HL + (s5 + 1) * 512)
                        for k in range(8):
                            nc.tensor.matmul(
                                psum_z[:, zcs],
                                lhsT=zw4[:, k * 128:(k + 1) * 128],
                                rhs=xT_sb[k][:, xcs],
                                start=(k == 0), stop=(k == 7),
                            )
                    nc.scalar.activation(
                        out=sz_full[:, seg * HL:(seg + 1) * HL],
                        in_=psum_z, func=AF.Silu)
                new_pending.append(lambda zrs=zrs, sz_full=sz_full: nc.sync.dma_start(
                    out=stage_d[zrs, SEQ:2 * SEQ], in_=sz_full))
                sz_fifo.append(sz_full[:, 0:HL])

            for b in range(PD):
                prefetch(b)

            for blk in range(NB):
                rs = slice(blk * 128, (blk + 1) * 128)
                xc_t, sz_t, dt_h, dtxc = fifo.pop(0)
                if blk + PD < NB:
                    prefetch(blk + PD)
                if half == 0:
                    emit_z(blk)
                for fn in pending:
                    fn()
                pending = list(new_pending)
                new_pending.clear()
                if half == 0:
                    sz_t = sz_fifo.pop(0)
                psum_y = ppy.tile([128, HL], f32, tag="py")
                # Pool u's first so Pool streams without bubbles
                u_t = {}
                for n in POOL_NS:
                    ns = slice(n * HL, (n + 1) * HL)
                    u = p5u.tile([128, HL], f16, tag="up")
                    nc.gpsimd.tensor_tensor(out=u, in0=dtxc, in1=B_bc[:, ns], op=OP.mult)
                    u_t[n] = u
                for i, n in enumerate(N_ORDER):
                    ns = slice(n * HL, (n + 1) * HL)
                    cc = blk * D_STATE + n
                    dA = p5dA.tile([128, HL], f32, tag="dA", bufs=3)
                    nc.scalar.activation(
                        out=dA, in_=dt_h, func=AF.Exp, scale=A_col(blk, n),
                    )
                    if n < ND:
                        u = p5u.tile([128, HL], f16, tag="u", bufs=2)
                        nc.vector.tensor_tensor(out=u, in0=dtxc, in1=B_bc[:, ns],
                                                op=OP.mult)
                    else:
                        u = u_t[n]
                    h = p5h.tile([128, HL], f16, tag="h")
                    init = 0.0 if half == 0 else carry_all[:, cc:cc + 1]
                    nc.vector.tensor_tensor_scan(
                        out=h, data0=dA, data1=u, initial=init,
                        op0=OP.mult, op1=OP.add,
                    )
                    if half == 0:
                        nc.vector.tensor_copy(
                            out=carry_all[:, cc:cc + 1], in_=h[:, HL - 1:HL])
                    hc = p5hc.tile([128, HL], f16, tag="hc")
                    eng = nc.vector if n < ND else nc.gpsimd
                    eng.tensor_tensor(out=hc, in0=h, in1=C_bc[:, ns], op=OP.mult)
                    for nt in range(2):
                        cs = slice(nt * 512, (nt + 1) * 512)
                        nc.tensor.matmul(
                            psum_y[:, cs], lhsT=I128, rhs=hc[:, cs],
                            start=(i == 0), stop=(i == D_STATE - 1),
                        )
                y1 = p5w.tile([128, HL], f16, tag="y1", bufs=3)
                nc.vector.scalar_tensor_tensor(
                    out=y1, in0=xc_t, scalar=Dp_col(blk), in1=psum_y,
                    op0=OP.mult, op1=OP.add,
                )
                yf = p5w.tile([128, HL], f16, tag="yf")
                nc.vector.tensor_tensor(out=yf, in0=y1, in1=sz_t, op=OP.mult)
                pending.append(lambda rs=rs, yf=yf, hs=hs: nc.sync.dma_start(
                    out=yf_d[rs, hs], in_=yf))
                if half == 1 and blk == 1:
                    emit_p6(0, p6y, ppo)
                if half == 1 and blk == 3:
                    emit_p6(1, p6y, ppo)
            for fn in pending:
                fn()

        with tc.tile_pool(name="bc0", bufs=1) as bc_pool, \
             tc.tile_pool(name="p5s0", bufs=3) as p5s, \
             tc.tile_pool(name="p5w0", bufs=3) as p5w, \
             tc.tile_pool(name="p5u0", bufs=7) as p5u, \
             tc.tile_pool(name="p5hc0", bufs=8) as p5hc, \
             tc.tile_pool(name="p5h0", bufs=5) as p5h, \
             tc.tile_pool(name="p5dA0", bufs=3) as p5dA, \
             tc.tile_pool(name="zw", bufs=2) as zwpool, \
             tc.tile_pool(name="szp", bufs=2) as szpool, \
             tc.tile_pool(name="ppy0", bufs=2, space="PSUM") as ppy, \
             tc.tile_pool(name="ppz", bufs=1, space="PSUM") as ppz:
            emit_half(0, bc_pool, p5s, p5w, p5dA, ppy, p5u, p5hc, p5h,
                      zwpool=zwpool, szpool=szpool, ppz=ppz)
        xtpool.release()

        owpool = tc.alloc_tile_pool(name="owpool", bufs=1)
        for k in range(NB):
            t = owpool.tile([128, D_MODEL], f16, tag=f"owT{k}", name=f"owT{k}")
            nc.scalar.dma_start(out=t, in_=owT[k * 128:(k + 1) * 128, :])
            owT_sb.append(t)

        with tc.tile_pool(name="bc1", bufs=1) as bc_pool, \
             tc.tile_pool(name="p5s1", bufs=2) as p5s, \
             tc.tile_pool(name="p5w1", bufs=3) as p5w, \
             tc.tile_pool(name="p5u1", bufs=7) as p5u, \
             tc.tile_pool(name="p5hc1", bufs=3) as p5hc, \
             tc.tile_pool(name="p5h1", bufs=5) as p5h, \
             tc.tile_pool(name="p5dA1", bufs=3) as p5dA, \
             tc.tile_pool(name="p6y", bufs=1) as p6y, \
             tc.tile_pool(name="ppy1", bufs=2, space="PSUM") as ppy, \
             tc.tile_pool(name="ppo", bufs=2, space="PSUM") as ppo:
            emit_half(1, bc_pool, p5s, p5w, p5dA, ppy, p5u, p5hc, p5h,
                      p6y=p6y, ppo=ppo)
            emit_p6(2, p6y, ppo)
            emit_p6(3, p6y, ppo)
        owpool.release()

    nc.finalize()
    return nc


def _get_nc():
    if "nc" not in _CACHE:
        _CACHE["nc"] = build_program()
    return _CACHE["nc"]


def kernel(x, in_proj_w, conv_w, conv_b, x_proj_w, dt_proj_w, dt_proj_b,
           A_log, D_param, out_proj_w):
    from concourse.bass_utils import run_bass_kernel_spmd

    nc = _get_nc()

    x = np.asarray(x)
    wk = {}
    for d in range(2):
        wk[d] = {
            "in_wT": np.ascontiguousarray(np.asarray(in_proj_w[d]).T).astype(np.float16),
            "xwT": _pad_xwT(np.asarray(x_proj_w[d])),
            "dtwT": np.ascontiguousarray(np.asarray(dt_proj_w[d]).T).astype(np.float16),
            "owT": np.ascontiguousarray(np.asarray(out_proj_w[d]).T).astype(np.float16),
            "consts_packed": _pack_consts(
                np.asarray(conv_w[d]).astype(np.float32),
                np.asarray(conv_b[d]).astype(np.float32),
                np.asarray(dt_proj_b[d]).astype(np.float32),
                np.asarray(D_param[d]).astype(np.float32),
                (-np.exp(np.asarray(A_log[d]))).astype(np.float32)),
        }

    in_maps = []
    for u in range(8):
        d, b = divmod(u, 4)
        xb = np.asarray(x[b])
        if d == 1:
            xb = xb[::-1]
        m = dict(wk[d])
        m["xT"] = np.ascontiguousarray(xb.T).astype(np.float16)
        in_maps.append(m)

    res = run_bass_kernel_spmd(nc, in_maps, core_ids=list(range(8))).results

    out = np.zeros((B_SZ, SEQ, D_MODEL), np.float32)
    for u in range(8):
        d, b = divmod(u, 4)
        yu = res[u]["y"].T  # [SEQ, D_MODEL]
        if d == 1:
            yu = yu[::-1]
        out[b] += yu
    return out.astype(np.float32)

